# revision 1
# baseline (speedup 1.0000x reference)
"""MultiLoraLinear Trainium2 kernel.

Problem: x [8, 2048, 4096] f32, adapter_ids [8] int, weight [16, 64, 4096] f32
         out[b] = x[b] @ weight[adapter_ids[b]].T         -> [8, 2048, 64] f32

Sharding: data-parallel over batch. B == n_cores == 8, so each NeuronCore owns
one batch element. The adapter gather (MoE routing) happens on host: each core
receives only the single [64, 4096] adapter it needs, pre-transposed/tiled.

Per-core compute: out [2048, 64] = x_b [2048, 4096] @ wT [4096, 64].
This is DMA-bound (32 MB of x per core / ~358 GB/s HBM ~= 90 us), so the
kernel keeps the fp32 matmul path (4 cyc/row, measured ~416 ns/MM -> 213 us
PE, PE-bound) OFF the critical path by using an exact bf16 hi/lo split with
fp32 PSUM accumulation:

    x = xh + xl, w = wh + wl (bf16 hi + bf16 residual)
    out ~= wh.x_hi + wl.x_hi + wh.x_lo          (lo.lo term ~2^-18, dropped)

measured rel err vs fp32 reference: 4.4e-06 (bf16 products are exact in the
PE's fp32 accumulate; error comes from the 16-bit effective mantissa of the
hi+lo pair and the dropped lo.lo term).

The PE contracts along the partition dim, so x is host-pre-tiled IN-major:
xhl[kc, p, c, h, s] (kc = K-chunk pair, p = IN%128 partition, c = chunk in
pair, h = hi/lo plane, s = sequence). Each K-chunk-pair is one fully
contiguous 4 MB DMA with 16 KB contiguous per partition line.

Matmuls: stationary = [wh | wl] column-packed [128, 128], moving = x chunk
[128, 512]. One stream of xh produces both wh.xh (PSUM rows 0:64) and wl.xh
(rows 64:128); a second 64-col pass accumulates wh.xl into rows 0:64. The
hi/lo fold is a single DVE add at the end. 256 bf16 MMs ~= 55 us << DMA.

Measured (512-rep hardware-loop wall-clock slope, 8 cores): ~111 us/rep.
Pure-DMA probe of the same traffic: ~98 us. PE-only probe: ~55 us.
"""

import numpy as np
import ml_dtypes

import concourse.bass as bass
import concourse.tile as tile
from concourse import mybir
from concourse import bass_utils

B, S, IN, OUT, L = 8, 2048, 4096, 64, 16
N_CORES = 8
P = 128
KO = IN // P     # 32 contraction chunks of 128
CH = 1           # K-chunks per DMA (1 MB per transfer: halves pipeline fill/drain)
NCH = KO // CH
S4 = S // 512    # moving-dim chunks of 512 (PSUM bank limit)

F32 = mybir.dt.float32
BF16 = mybir.dt.bfloat16


def _split_sync_waits(nc):
    """walrus in this image supports very few sem-wait slots per instruction
    (fp32 Matmult rejects even 2). Move excess waits onto InstEventSemaphore
    carriers inserted immediately before the instruction on the same engine —
    same program point, so ordering semantics are unchanged."""
    counter = [0]

    def _carrier(engine, wait):
        counter[0] += 1
        e = mybir.InstEventSemaphore(name=f"wsplit-{counter[0]}", ins=[], outs=[])
        e.engine = engine
        e.sync_info = mybir.SyncInfo(on_wait=[wait], on_update=[])
        return e

    for f in nc.m.functions:
        for bb in f.blocks:
            new_insts = []
            for inst in bb.instructions:
                si = inst.sync_info
                waits = list(si.on_wait) if si and si.on_wait else []
                cap = 0 if isinstance(inst, mybir.InstMatmult) else 1
                if len(waits) > cap:
                    keep = waits[:cap]
                    for w in waits[cap:]:
                        c = _carrier(inst.engine, w)
                        nc.register_instruction(c, overwrite=True)
                        new_insts.append(c)
                    inst.sync_info = mybir.SyncInfo(
                        on_wait=keep, on_update=list(si.on_update or [])
                    )
                new_insts.append(inst)
            bb.instructions[:] = new_insts


def build_nc(n_rep: int = 1, x_bufs: int = 4):
    """Build the per-core Bass program. n_rep > 1 wraps the computation in a
    hardware For_i loop (same I/O, output overwritten) so harnesses can
    measure steady-state HW time by wall-clock slope; grading uses n_rep=1."""
    nc = bass.Bass("TRN2", target_bir_lowering=False, debug=False)
    x_ap = nc.dram_tensor("xhl", [NCH, P, CH, 2, S], BF16, kind="ExternalInput").ap()
    w_ap = nc.dram_tensor("wt", [P, KO, 2, OUT], BF16, kind="ExternalInput").ap()
    o_ap = nc.dram_tensor("out", [OUT, S], F32, kind="ExternalOutput").ap()

    with tile.TileContext(nc) as tc:
        with (
            tc.tile_pool(name="wpool", bufs=1) as wpool,
            tc.tile_pool(name="xpool", bufs=x_bufs) as xpool,
            tc.tile_pool(name="opool", bufs=2) as opool,
            tc.tile_pool(name="pspool", bufs=1, space="PSUM") as pspool,
        ):
            w_sb = wpool.tile([P, KO, 2, OUT], BF16)
            # SWDGE ring for the 1 MB weight preload so the x stream starts
            # immediately on the HWDGE ring.
            nc.gpsimd.dma_start(w_sb[:], w_ap[:])

            def body():
                pss = [
                    pspool.tile([P, 512], F32, tag=f"ps{s4}", name=f"ps{s4}")
                    for s4 in range(S4)
                ]
                for kc in range(NCH):
                    xt = xpool.tile([P, CH, 2, S], BF16, tag="xhl")
                    nc.sync.dma_start(xt[:], x_ap[kc])
                    for c in range(CH):
                        ko = kc * CH + c
                        w_pk = w_sb[:, ko, :, :]   # [128, 2*OUT] packed [wh|wl]
                        w_hi = w_sb[:, ko, 0, :]   # [128, OUT]
                        for s4 in range(S4):
                            xs_h = xt[:, c, 0, s4 * 512:(s4 + 1) * 512]
                            xs_l = xt[:, c, 1, s4 * 512:(s4 + 1) * 512]
                            nc.tensor.matmul(
                                pss[s4][:, :], w_pk, xs_h,
                                start=(ko == 0), stop=False,
                                skip_group_check=True,
                            )
                            nc.tensor.matmul(
                                pss[s4][:OUT, :], w_hi, xs_l,
                                start=False, stop=(ko == KO - 1),
                                skip_group_check=True,
                            )
                for s4 in range(S4):
                    ot = opool.tile([OUT, 512], F32, tag="ot")
                    nc.scalar.copy(ot[:], pss[s4][OUT:, :])
                    nc.vector.tensor_add(ot[:], ot[:], pss[s4][:OUT, :])
                    nc.sync.dma_start(o_ap[:, s4 * 512:(s4 + 1) * 512], ot[:])

            if n_rep == 1:
                body()
            else:
                with tc.For_i(0, n_rep, 1):
                    body()
    _split_sync_waits(nc)
    return nc


def make_in_maps(x: np.ndarray, adapter_ids: np.ndarray, weight: np.ndarray):
    """Host-side sharding: per-core adapter gather + bf16 hi/lo split + tiling.

    xhl[kc, p, c, h, s] = split(x[b, s, (kc*CH+c)*128 + p])[h]
    wt[p, ko, h, o]     = split(weight[id_b, o, ko*128 + p])[h]
    """
    x = np.asarray(x, dtype=np.float32)
    ids = np.asarray(adapter_ids).astype(np.int64)
    w = np.asarray(weight, dtype=np.float32)

    # vectorized across the batch: one transpose + one bf16 split for all cores
    xa = np.ascontiguousarray(x.transpose(0, 2, 1)).reshape(B, KO, P, S)
    xh = xa.astype(ml_dtypes.bfloat16)
    xl = (xa - xh.astype(np.float32)).astype(ml_dtypes.bfloat16)
    xhl = np.stack([xh, xl], axis=2)                       # [B, KO, 2, P, S]
    xhl = np.ascontiguousarray(
        xhl.reshape(B, NCH, CH, 2, P, S).transpose(0, 1, 4, 2, 3, 5)
    )                                                      # [B, NCH, P, CH, 2, S]

    wsel = w[ids]                                          # [B, OUT, IN]
    wt = np.ascontiguousarray(wsel.transpose(0, 2, 1)).reshape(B, KO, P, OUT)
    wt = wt.transpose(0, 2, 1, 3)                          # [B, P, KO, OUT]
    wh = wt.astype(ml_dtypes.bfloat16)
    wl = (wt - wh.astype(np.float32)).astype(ml_dtypes.bfloat16)
    wpk = np.ascontiguousarray(np.stack([wh, wl], axis=3))  # [B, P, KO, 2, OUT]

    return [{"xhl": xhl[b], "wt": wpk[b]} for b in range(B)]


_NC_CACHE = {}


def kernel(x, adapter_ids, weight):
    x = np.asarray(x)
    assert x.shape == (B, S, IN), x.shape
    if "nc" not in _NC_CACHE:
        _NC_CACHE["nc"] = build_nc()
    nc = _NC_CACHE["nc"]
    in_maps = make_in_maps(x, adapter_ids, weight)
    res = bass_utils.run_bass_kernel_spmd(
        nc, in_maps, core_ids=list(range(N_CORES)), trace=False
    )
    out = np.stack(
        [res.results[b]["out"].T for b in range(B)], axis=0
    )
    return np.ascontiguousarray(out, dtype=np.float32)



# revision 4
# speedup vs baseline: 1.6104x; 1.6104x over previous
"""MultiLoraLinear Trainium2 kernel — bf16 single-plane variant.

Problem: x [8, 2048, 4096] f32, adapter_ids [8] int, weight [16, 64, 4096] f32
         out[b] = x[b] @ weight[adapter_ids[b]].T         -> [8, 2048, 64] f32

Sharding: data-parallel over batch. B == n_cores == 8, so each NeuronCore owns
one batch element; the adapter gather happens on host (each core receives only
the [64, 4096] adapter it needs, pre-transposed/tiled).

The kernel is DMA-bound. The correctness gate is rel_err < 2e-2; rounding both
x and w to bf16 gives rel err ~1.6e-3 (per-element RMS rel err 2^-9/sqrt(3)
for each operand, errors of the 4096 accumulated products are independent so
the result's relative error matches the per-product error). This halves the
x traffic vs the fp32-exact hi/lo-split baseline: 16 MB of x per core
(+0.5 MB w, +0.5 MB out) at ~326-358 GB/s/core -> ~50 us floor.

PE: 128 bf16 matmuls [K=128 x M=64 x N=512] ~= 215 ns each ~= 27.5 us,
fully hidden under the DMA stream.

Layout: x pre-tiled IN-major on host: xh[kc, p, s] = bf16(x[b, s, kc*128+p]),
one fully contiguous 512 KB DMA per K-chunk (4 KB per partition line).
"""

import numpy as np
import ml_dtypes

import concourse.bass as bass
import concourse.tile as tile
from concourse import mybir
from concourse import bass_utils

B, S, IN, OUT, L = 8, 2048, 4096, 64, 16
N_CORES = 8
P = 128
KO = IN // P     # 32 contraction chunks of 128
CH = 1           # K-chunks per DMA transfer
NCH = KO // CH
S4 = S // 512    # moving-dim chunks of 512 (PSUM bank limit)

F32 = mybir.dt.float32
BF16 = mybir.dt.bfloat16

DUAL_QUEUE = False   # alternate x DMAs between the SP and Act HWDGE queues


def _split_sync_waits(nc):
    """walrus in this image supports very few sem-wait slots per instruction
    (Matmult rejects even 2). Move excess waits onto InstEventSemaphore
    carriers inserted immediately before the instruction on the same engine —
    same program point, so ordering semantics are unchanged."""
    counter = [0]

    def _carrier(engine, wait):
        counter[0] += 1
        e = mybir.InstEventSemaphore(name=f"wsplit-{counter[0]}", ins=[], outs=[])
        e.engine = engine
        e.sync_info = mybir.SyncInfo(on_wait=[wait], on_update=[])
        return e

    for f in nc.m.functions:
        for bb in f.blocks:
            new_insts = []
            for inst in bb.instructions:
                si = inst.sync_info
                waits = list(si.on_wait) if si and si.on_wait else []
                cap = 0 if isinstance(inst, mybir.InstMatmult) else 1
                if len(waits) > cap:
                    keep = waits[:cap]
                    for w in waits[cap:]:
                        c = _carrier(inst.engine, w)
                        nc.register_instruction(c, overwrite=True)
                        new_insts.append(c)
                    inst.sync_info = mybir.SyncInfo(
                        on_wait=keep, on_update=list(si.on_update or [])
                    )
                new_insts.append(inst)
            bb.instructions[:] = new_insts


def build_nc(n_rep: int = 1, x_bufs: int = 4, dual_queue: bool = None,
             s_split: int = 1):
    """Build the per-core Bass program. n_rep > 1 wraps the computation in a
    hardware For_i loop (same I/O, output overwritten) so harnesses can
    measure steady-state HW time by wall-clock slope; grading uses n_rep=1."""
    if dual_queue is None:
        dual_queue = DUAL_QUEUE
    nc = bass.Bass("TRN2", target_bir_lowering=False, debug=False)
    x_ap = nc.dram_tensor("xh", [NCH, P, CH * S], BF16, kind="ExternalInput").ap()
    w_ap = nc.dram_tensor("wt", [P, KO, OUT], BF16, kind="ExternalInput").ap()
    o_ap = nc.dram_tensor("out", [OUT, S], F32, kind="ExternalOutput").ap()

    with tile.TileContext(nc) as tc:
        with (
            tc.tile_pool(name="wpool", bufs=1) as wpool,
            tc.tile_pool(name="xpool", bufs=x_bufs) as xpool,
            tc.tile_pool(name="opool", bufs=2) as opool,
            tc.tile_pool(name="pspool", bufs=1, space="PSUM") as pspool,
        ):
            w_sb = wpool.tile([P, KO, OUT], BF16)
            # SWDGE ring for the 0.5 MB weight preload so the x stream starts
            # immediately on the HWDGE ring.
            nc.gpsimd.dma_start(w_sb[:], w_ap[:])

            def body():
                pss = [
                    pspool.tile([OUT, 512], F32, tag=f"ps{s4}", name=f"ps{s4}")
                    for s4 in range(S4)
                ]
                for kc in range(NCH):
                    xt = xpool.tile([P, CH * S], BF16, tag="xh")
                    if s_split == 1:
                        eng = nc.scalar if (dual_queue and kc % 2) else nc.sync
                        eng.dma_start(xt[:], x_ap[kc])
                    else:
                        w_s = (CH * S) // s_split
                        for j in range(s_split):
                            eng = (nc.scalar
                                   if (dual_queue and (kc * s_split + j) % 2)
                                   else nc.sync)
                            eng.dma_start(xt[:, j * w_s:(j + 1) * w_s],
                                          x_ap[kc][:, j * w_s:(j + 1) * w_s])
                    for c in range(CH):
                        ko = kc * CH + c
                        for s4 in range(S4):
                            xs = xt[:, c * S + s4 * 512:c * S + (s4 + 1) * 512]
                            nc.tensor.matmul(
                                pss[s4][:, :], w_sb[:, ko, :], xs,
                                start=(ko == 0), stop=(ko == KO - 1),
                                skip_group_check=True,
                            )
                for s4 in range(S4):
                    ot = opool.tile([OUT, 512], F32, tag="ot")
                    nc.scalar.copy(ot[:], pss[s4][:])
                    nc.sync.dma_start(o_ap[:, s4 * 512:(s4 + 1) * 512], ot[:])

            if n_rep == 1:
                body()
            else:
                with tc.For_i(0, n_rep, 1):
                    body()
    _split_sync_waits(nc)
    return nc


def make_in_maps(x: np.ndarray, adapter_ids: np.ndarray, weight: np.ndarray):
    """Host-side sharding: per-core adapter gather + bf16 round + IN-major tiling.

    xh[kc, p, c*S + s] = bf16(x[b, s, (kc*CH+c)*128 + p])
    wt[p, ko, o]       = bf16(weight[id_b, o, ko*128 + p])
    """
    x = np.asarray(x, dtype=np.float32)
    ids = np.asarray(adapter_ids).astype(np.int64)
    w = np.asarray(weight, dtype=np.float32)

    xa = np.ascontiguousarray(x.transpose(0, 2, 1)).reshape(B, KO, P, S)
    xh = xa.astype(ml_dtypes.bfloat16)                     # [B, KO, P, S]
    if CH > 1:
        xh = np.ascontiguousarray(
            xh.reshape(B, NCH, CH, P, S).transpose(0, 1, 3, 2, 4)
        )                                                  # [B, NCH, P, CH, S]
    xh = xh.reshape(B, NCH, P, CH * S)

    wsel = w[ids]                                          # [B, OUT, IN]
    wt = np.ascontiguousarray(wsel.transpose(0, 2, 1)).reshape(B, KO, P, OUT)
    wt = np.ascontiguousarray(wt.transpose(0, 2, 1, 3))    # [B, P, KO, OUT]
    wh = wt.astype(ml_dtypes.bfloat16)

    return [{"xh": xh[b], "wt": wh[b]} for b in range(B)]


_NC_CACHE = {}


def kernel(x, adapter_ids, weight):
    x = np.asarray(x)
    assert x.shape == (B, S, IN), x.shape
    if "nc" not in _NC_CACHE:
        _NC_CACHE["nc"] = build_nc()
    nc = _NC_CACHE["nc"]
    in_maps = make_in_maps(x, adapter_ids, weight)
    res = bass_utils.run_bass_kernel_spmd(
        nc, in_maps, core_ids=list(range(N_CORES)), trace=False
    )
    out = np.stack(
        [res.results[b]["out"].T for b in range(B)], axis=0
    )
    return np.ascontiguousarray(out, dtype=np.float32)


# revision 5
# speedup vs baseline: 1.6875x; 1.0478x over previous
"""MultiLoraLinear Trainium2 kernel — bf16, s-major big-line DMA variant.

Problem: x [8, 2048, 4096] f32, adapter_ids [8] int, weight [16, 64, 4096] f32
         out[b] = x[b] @ weight[adapter_ids[b]].T         -> [8, 2048, 64] f32

Sharding: data-parallel over batch. B == n_cores == 8, so each NeuronCore owns
one batch element; the adapter gather happens on host (each core receives only
the [64, 4096] adapter it needs, pre-transposed/tiled).

Precision: the correctness gate is rel_err < 2e-2; rounding both x and w to
bf16 gives rel err ~2e-3 (per-element RMS rel err 2^-9/sqrt(3) per operand;
the 4096 accumulated per-product errors are independent). This halves x
traffic vs an exact hi/lo split: 16 MB of x per core.

DMA: empirically the per-core x stream rate is set by partition-line size
(~9 ns fixed + bytes/~500 GB/s per line-descriptor, ~320 GB/s at 8 KB lines,
~240 GB/s at 4 KB) plus ~565 ns of queue sequencing per dma_start. So x is
laid out s-major: one DMA per 256-column s-slice carries all 32 K-chunks as a
single 2 MB transfer with 16 KB contiguous per partition line — 8 transfers
total, ~350+ GB/s.

Compute: per s-slice, 32 bf16 matmuls [K=128 x M=64 x N=256] accumulate in
one PSUM bank (~3.5 us) while the next slice's DMA (~5.7 us) streams: PE
stays off the critical path. Output folds into a persistent SBUF tile; one
early DMA for s-slices 0..6 overlaps the last slice's matmuls, then a final
256-column DMA is the only tail.
"""

import numpy as np
import ml_dtypes

import concourse.bass as bass
import concourse.tile as tile
from concourse import mybir
from concourse import bass_utils

B, S, IN, OUT, L = 8, 2048, 4096, 64, 16
N_CORES = 8
P = 128
KO = IN // P     # 32 contraction chunks of 128
J = 8            # s-slices (one DMA each); SS = S // J columns per slice

F32 = mybir.dt.float32
BF16 = mybir.dt.bfloat16


def set_J(j: int):
    global J
    J = j


def _split_sync_waits(nc):
    """walrus in this image supports very few sem-wait slots per instruction
    (Matmult rejects even 2). Move excess waits onto InstEventSemaphore
    carriers inserted immediately before the instruction on the same engine —
    same program point, so ordering semantics are unchanged."""
    counter = [0]

    def _carrier(engine, wait):
        counter[0] += 1
        e = mybir.InstEventSemaphore(name=f"wsplit-{counter[0]}", ins=[], outs=[])
        e.engine = engine
        e.sync_info = mybir.SyncInfo(on_wait=[wait], on_update=[])
        return e

    for f in nc.m.functions:
        for bb in f.blocks:
            new_insts = []
            for inst in bb.instructions:
                si = inst.sync_info
                waits = list(si.on_wait) if si and si.on_wait else []
                cap = 0 if isinstance(inst, mybir.InstMatmult) else 1
                if len(waits) > cap:
                    keep = waits[:cap]
                    for w in waits[cap:]:
                        c = _carrier(inst.engine, w)
                        nc.register_instruction(c, overwrite=True)
                        new_insts.append(c)
                    inst.sync_info = mybir.SyncInfo(
                        on_wait=keep, on_update=list(si.on_update or [])
                    )
                new_insts.append(inst)
            bb.instructions[:] = new_insts


def build_nc(n_rep: int = 1, x_bufs: int = 3, out_split: bool = True):
    """Build the per-core Bass program. n_rep > 1 wraps the computation in a
    hardware For_i loop (same I/O, output overwritten) so harnesses can
    measure steady-state HW time by wall-clock slope; grading uses n_rep=1."""
    SS = S // J
    nc = bass.Bass("TRN2", target_bir_lowering=False, debug=False)
    x_ap = nc.dram_tensor("xh", [J, P, KO * SS], BF16, kind="ExternalInput").ap()
    w_ap = nc.dram_tensor("wt", [P, KO, OUT], BF16, kind="ExternalInput").ap()
    o_ap = nc.dram_tensor("out", [OUT, S], F32, kind="ExternalOutput").ap()

    with tile.TileContext(nc) as tc:
        with (
            tc.tile_pool(name="wpool", bufs=1) as wpool,
            tc.tile_pool(name="xpool", bufs=x_bufs) as xpool,
            tc.tile_pool(name="opool", bufs=1) as opool,
            tc.tile_pool(name="pspool", bufs=2, space="PSUM") as pspool,
        ):
            w_sb = wpool.tile([P, KO, OUT], BF16)
            # SWDGE ring for the 0.5 MB weight preload so the x stream starts
            # immediately on the qSP HWDGE ring.
            nc.gpsimd.dma_start(w_sb[:], w_ap[:])

            def body():
                osb = opool.tile([OUT, S], F32, tag="osb")
                for j in range(J):
                    xt = xpool.tile([P, KO * SS], BF16, tag="xh")
                    nc.sync.dma_start(xt[:], x_ap[j])
                    ps = pspool.tile([OUT, 512], F32, tag="ps")
                    for ko in range(KO):
                        nc.tensor.matmul(
                            ps[:, :SS], w_sb[:, ko, :],
                            xt[:, ko * SS:(ko + 1) * SS],
                            start=(ko == 0), stop=(ko == KO - 1),
                            skip_group_check=True,
                        )
                    nc.scalar.copy(osb[:, j * SS:(j + 1) * SS], ps[:, :SS])
                    # overlap most of the output store with the last slice's
                    # matmuls; only the final SS columns are a tail. Act-queue
                    # DMAs so the x stream on qSP is never interrupted.
                    if out_split and j == J - 2:
                        nc.scalar.dma_start(
                            o_ap[:, :(J - 1) * SS], osb[:, :(J - 1) * SS]
                        )
                if out_split:
                    nc.scalar.dma_start(
                        o_ap[:, (J - 1) * SS:], osb[:, (J - 1) * SS:]
                    )
                else:
                    nc.scalar.dma_start(o_ap[:], osb[:])

            if n_rep == 1:
                body()
            else:
                with tc.For_i(0, n_rep, 1):
                    body()
    _split_sync_waits(nc)
    return nc


def make_in_maps(x: np.ndarray, adapter_ids: np.ndarray, weight: np.ndarray):
    """Host-side sharding: per-core adapter gather + bf16 round + s-major tiling.

    xh[j, p, ko*SS + s] = bf16(x[b, j*SS + s, ko*128 + p])
    wt[p, ko, o]        = bf16(weight[id_b, o, ko*128 + p])
    """
    SS = S // J
    x = np.asarray(x, dtype=np.float32)
    ids = np.asarray(adapter_ids).astype(np.int64)
    w = np.asarray(weight, dtype=np.float32)

    xa = np.ascontiguousarray(x.transpose(0, 2, 1))        # [B, IN, S]
    xh = xa.astype(ml_dtypes.bfloat16).reshape(B, KO, P, J, SS)
    xh = np.ascontiguousarray(xh.transpose(0, 3, 2, 1, 4)) # [B, J, P, KO, SS]
    xh = xh.reshape(B, J, P, KO * SS)

    wsel = w[ids]                                          # [B, OUT, IN]
    wt = np.ascontiguousarray(wsel.transpose(0, 2, 1)).reshape(B, KO, P, OUT)
    wt = np.ascontiguousarray(wt.transpose(0, 2, 1, 3))    # [B, P, KO, OUT]
    wh = wt.astype(ml_dtypes.bfloat16)

    return [{"xh": xh[b], "wt": wh[b]} for b in range(B)]


_NC_CACHE = {}


def kernel(x, adapter_ids, weight):
    x = np.asarray(x)
    assert x.shape == (B, S, IN), x.shape
    if "nc" not in _NC_CACHE:
        _NC_CACHE["nc"] = build_nc()
    nc = _NC_CACHE["nc"]
    in_maps = make_in_maps(x, adapter_ids, weight)
    res = bass_utils.run_bass_kernel_spmd(
        nc, in_maps, core_ids=list(range(N_CORES)), trace=False
    )
    out = np.stack(
        [res.results[b]["out"].T for b in range(B)], axis=0
    )
    return np.ascontiguousarray(out, dtype=np.float32)


# revision 12
# speedup vs baseline: 1.7427x; 1.0327x over previous
"""MultiLoraLinear Trainium2 kernel — bf16, s-major big-line DMA variant.

Problem: x [8, 2048, 4096] f32, adapter_ids [8] int, weight [16, 64, 4096] f32
         out[b] = x[b] @ weight[adapter_ids[b]].T         -> [8, 2048, 64] f32

Sharding: data-parallel over batch. B == n_cores == 8, so each NeuronCore owns
one batch element; the adapter gather happens on host (each core receives only
the [64, 4096] adapter it needs, pre-transposed/tiled).

Precision: the correctness gate is rel_err < 2e-2; rounding both x and w to
bf16 gives rel err ~2e-3 (per-element RMS rel err 2^-9/sqrt(3) per operand;
the 4096 accumulated per-product errors are independent). This halves x
traffic vs an exact hi/lo split: 16 MB of x per core.

DMA: empirically the per-core x stream rate is set by partition-line size
(~9 ns fixed + bytes/~500 GB/s per line-descriptor, ~320 GB/s at 8 KB lines,
~240 GB/s at 4 KB) plus ~565 ns of queue sequencing per dma_start. So x is
laid out s-major: one DMA per 256-column s-slice carries all 32 K-chunks as a
single 2 MB transfer with 16 KB contiguous per partition line — 8 transfers
total, ~350+ GB/s.

Compute: per s-slice, 32 bf16 matmuls [K=128 x M=64 x N=256] accumulate in
one PSUM bank (~3.5 us) while the next slice's DMA (~5.7 us) streams: PE
stays off the critical path. Output folds into a persistent SBUF tile; one
early DMA for s-slices 0..6 overlaps the last slice's matmuls, then a final
256-column DMA is the only tail.
"""

import numpy as np
import ml_dtypes

import concourse.bass as bass
import concourse.tile as tile
from concourse import mybir
from concourse import bass_utils

B, S, IN, OUT, L = 8, 2048, 4096, 64, 16
N_CORES = 8
P = 128
KO = IN // P     # 32 contraction chunks of 128
J = 8            # s-slices (one DMA each); SS = S // J columns per slice

F32 = mybir.dt.float32
BF16 = mybir.dt.bfloat16


def set_J(j: int):
    global J
    J = j


def _split_sync_waits(nc):
    """walrus in this image supports very few sem-wait slots per instruction
    (Matmult rejects even 2). Move excess waits onto InstEventSemaphore
    carriers inserted immediately before the instruction on the same engine —
    same program point, so ordering semantics are unchanged."""
    counter = [0]

    def _carrier(engine, wait):
        counter[0] += 1
        e = mybir.InstEventSemaphore(name=f"wsplit-{counter[0]}", ins=[], outs=[])
        e.engine = engine
        e.sync_info = mybir.SyncInfo(on_wait=[wait], on_update=[])
        return e

    for f in nc.m.functions:
        for bb in f.blocks:
            new_insts = []
            for inst in bb.instructions:
                si = inst.sync_info
                waits = list(si.on_wait) if si and si.on_wait else []
                cap = 0 if isinstance(inst, mybir.InstMatmult) else 1
                if len(waits) > cap:
                    keep = waits[:cap]
                    for w in waits[cap:]:
                        c = _carrier(inst.engine, w)
                        nc.register_instruction(c, overwrite=True)
                        new_insts.append(c)
                    inst.sync_info = mybir.SyncInfo(
                        on_wait=keep, on_update=list(si.on_update or [])
                    )
                new_insts.append(inst)
            bb.instructions[:] = new_insts


def build_nc(n_rep: int = 1, x_bufs: int = 4, out_split: bool = True,
             dummies: bool = True, last_kspl: int = 2, dual: bool = False,
             dummy_n: int = 36, pre_dummy_n: int = 52, out_bf16: bool = False):
    """Build the per-core Bass program. n_rep > 1 wraps the computation in a
    hardware For_i loop (same I/O, output overwritten) so harnesses can
    measure steady-state HW time by wall-clock slope; grading uses n_rep=1.

    dummies: fill PE idle gaps between s-slices with throwaway matmuls into a
      scratch PSUM bank so the Tensor engine never idles and stays at its max
      pstate (it runs at ~2x cycle time until ~3 us of CONTINUOUS execution;
      DMA-paced bursts of ~3.4 us never ramp up otherwise).
    last_kspl: split the last s-slice's DMA into this many K-range pieces so
      its matmuls overlap the stream and only the last piece is a PE tail.
    dual: alternate s-slice DMAs between the SP and Act HWDGE queues.
    """
    SS = S // J
    ODT = BF16 if out_bf16 else F32
    nc = bass.Bass("TRN2", target_bir_lowering=False, debug=False)
    x_ap = nc.dram_tensor("xh", [J, P, KO * SS], BF16, kind="ExternalInput").ap()
    w_ap = nc.dram_tensor("wt", [P, KO, OUT], BF16, kind="ExternalInput").ap()
    o_ap = nc.dram_tensor("out", [OUT, S], ODT, kind="ExternalOutput").ap()

    with tile.TileContext(nc) as tc:
        with (
            tc.tile_pool(name="wpool", bufs=1) as wpool,
            tc.tile_pool(name="xpool", bufs=x_bufs) as xpool,
            tc.tile_pool(name="opool", bufs=1) as opool,
            tc.tile_pool(name="pspool", bufs=2, space="PSUM") as pspool,
            tc.tile_pool(name="jkpool", bufs=1, space="PSUM") as jkpool,
        ):
            w_sb = wpool.tile([P, KO, OUT], BF16)
            # SWDGE ring for the 0.5 MB weight preload so the x stream starts
            # immediately on the qSP HWDGE ring.
            nc.gpsimd.dma_start(w_sb[:], w_ap[:])
            ps_junk = jkpool.tile([OUT, 512], F32, tag="junk")

            def emit_dummies(n):
                # reads of resident w_sb; write-only scratch PSUM; no
                # cross-engine deps, so these run back-to-back on the PE
                # filling what would otherwise be an idle (pstate-dropping)
                # gap, and the next real matmul preempts them at the next
                # instruction boundary.
                for d in range(n):
                    nc.tensor.matmul(
                        ps_junk[:, :OUT], w_sb[:, 0, :],
                        w_sb[:, 1 + (d % (KO - 1)), :],
                        start=True, stop=True, skip_group_check=True,
                    )

            def body():
                osb = opool.tile([OUT, S], ODT, tag="osb")
                if dummies:
                    emit_dummies(pre_dummy_n)
                for j in range(J):
                    xt = xpool.tile([P, KO * SS], BF16, tag="xh")
                    nspl = last_kspl if j == J - 1 else 1
                    kh = KO // nspl
                    eng = nc.scalar if (dual and j % 2) else nc.sync
                    for h in range(nspl):
                        eng.dma_start(
                            xt[:, h * kh * SS:(h + 1) * kh * SS],
                            x_ap[j][:, h * kh * SS:(h + 1) * kh * SS],
                        )
                    ps = pspool.tile([OUT, 512], F32, tag="ps")
                    for ko in range(KO):
                        nc.tensor.matmul(
                            ps[:, :SS], w_sb[:, ko, :],
                            xt[:, ko * SS:(ko + 1) * SS],
                            start=(ko == 0), stop=(ko == KO - 1),
                            skip_group_check=True,
                        )
                    nc.scalar.copy(osb[:, j * SS:(j + 1) * SS], ps[:, :SS])
                    if dummies and j < J - 1:
                        emit_dummies(dummy_n)
                    # overlap most of the output store with the last slice's
                    # matmuls; only the final SS columns are a tail. (SWDGE
                    # can't encode a strided DRAM dst - "ISA wrong length" -
                    # so these ride the Act HWDGE queue.)
                    if out_split and j == J - 2:
                        nc.scalar.dma_start(
                            o_ap[:, :(J - 1) * SS], osb[:, :(J - 1) * SS]
                        )
                if out_split:
                    nc.scalar.dma_start(
                        o_ap[:, (J - 1) * SS:], osb[:, (J - 1) * SS:]
                    )
                else:
                    nc.scalar.dma_start(o_ap[:], osb[:])

            if n_rep == 1:
                body()
            else:
                with tc.For_i(0, n_rep, 1):
                    body()
    _split_sync_waits(nc)
    return nc


def make_in_maps(x: np.ndarray, adapter_ids: np.ndarray, weight: np.ndarray):
    """Host-side sharding: per-core adapter gather + bf16 round + s-major tiling.

    xh[j, p, ko*SS + s] = bf16(x[b, j*SS + s, ko*128 + p])
    wt[p, ko, o]        = bf16(weight[id_b, o, ko*128 + p])
    """
    SS = S // J
    x = np.asarray(x, dtype=np.float32)
    ids = np.asarray(adapter_ids).astype(np.int64)
    w = np.asarray(weight, dtype=np.float32)

    xa = np.ascontiguousarray(x.transpose(0, 2, 1))        # [B, IN, S]
    xh = xa.astype(ml_dtypes.bfloat16).reshape(B, KO, P, J, SS)
    xh = np.ascontiguousarray(xh.transpose(0, 3, 2, 1, 4)) # [B, J, P, KO, SS]
    xh = xh.reshape(B, J, P, KO * SS)

    wsel = w[ids]                                          # [B, OUT, IN]
    wt = np.ascontiguousarray(wsel.transpose(0, 2, 1)).reshape(B, KO, P, OUT)
    wt = np.ascontiguousarray(wt.transpose(0, 2, 1, 3))    # [B, P, KO, OUT]
    wh = wt.astype(ml_dtypes.bfloat16)

    return [{"xh": xh[b], "wt": wh[b]} for b in range(B)]


_NC_CACHE = {}


def kernel(x, adapter_ids, weight):
    x = np.asarray(x)
    assert x.shape == (B, S, IN), x.shape
    if "nc" not in _NC_CACHE:
        _NC_CACHE["nc"] = build_nc()
    nc = _NC_CACHE["nc"]
    in_maps = make_in_maps(x, adapter_ids, weight)
    res = bass_utils.run_bass_kernel_spmd(
        nc, in_maps, core_ids=list(range(N_CORES)), trace=False
    )
    out = np.stack(
        [res.results[b]["out"].T for b in range(B)], axis=0
    )
    return np.ascontiguousarray(out, dtype=np.float32)


# revision 18
# speedup vs baseline: 2.0149x; 1.1562x over previous
"""MultiLoraLinear Trainium2 kernel — bf16, s-major big-line DMA variant.

Problem: x [8, 2048, 4096] f32, adapter_ids [8] int, weight [16, 64, 4096] f32
         out[b] = x[b] @ weight[adapter_ids[b]].T         -> [8, 2048, 64] f32

Sharding: data-parallel over batch. B == n_cores == 8, so each NeuronCore owns
one batch element; the adapter gather happens on host (each core receives only
the [64, 4096] adapter it needs, pre-transposed/tiled).

Precision: the correctness gate is rel_err < 2e-2; rounding both x and w to
bf16 gives rel err ~2e-3 (per-element RMS rel err 2^-9/sqrt(3) per operand;
the 4096 accumulated per-product errors are independent). This halves x
traffic vs an exact hi/lo split: 16 MB of x per core.

DMA: empirically the per-core x stream rate is set by partition-line size
(~9 ns fixed + bytes/~500 GB/s per line-descriptor, ~320 GB/s at 8 KB lines,
~240 GB/s at 4 KB) plus ~565 ns of queue sequencing per dma_start. So x is
laid out s-major: one DMA per 256-column s-slice carries all 32 K-chunks as a
single 2 MB transfer with 16 KB contiguous per partition line — 8 transfers
total, ~350+ GB/s.

Compute: per s-slice, 32 bf16 matmuls [K=128 x M=64 x N=256] accumulate in
one PSUM bank (~3.5 us) while the next slice's DMA (~5.7 us) streams: PE
stays off the critical path. Output folds into a persistent SBUF tile; one
early DMA for s-slices 0..6 overlaps the last slice's matmuls, then a final
256-column DMA is the only tail.
"""

import numpy as np
import ml_dtypes

import concourse.bass as bass
import concourse.tile as tile
from concourse import mybir
from concourse import bass_utils

B, S, IN, OUT, L = 8, 2048, 4096, 64, 16
N_CORES = 8
P = 128
KO = IN // P     # 32 contraction chunks of 128
J = 8            # s-slices (one DMA each); SS = S // J columns per slice

F32 = mybir.dt.float32
BF16 = mybir.dt.bfloat16


def set_J(j: int):
    global J
    J = j


def _split_sync_waits(nc):
    """walrus in this image supports very few sem-wait slots per instruction
    (Matmult rejects even 2). Move excess waits onto InstEventSemaphore
    carriers inserted immediately before the instruction on the same engine —
    same program point, so ordering semantics are unchanged."""
    counter = [0]

    def _carrier(engine, wait):
        counter[0] += 1
        e = mybir.InstEventSemaphore(name=f"wsplit-{counter[0]}", ins=[], outs=[])
        e.engine = engine
        e.sync_info = mybir.SyncInfo(on_wait=[wait], on_update=[])
        return e

    for f in nc.m.functions:
        for bb in f.blocks:
            new_insts = []
            for inst in bb.instructions:
                si = inst.sync_info
                waits = list(si.on_wait) if si and si.on_wait else []
                cap = 0 if isinstance(inst, mybir.InstMatmult) else 1
                if len(waits) > cap:
                    keep = waits[:cap]
                    for w in waits[cap:]:
                        c = _carrier(inst.engine, w)
                        nc.register_instruction(c, overwrite=True)
                        new_insts.append(c)
                    inst.sync_info = mybir.SyncInfo(
                        on_wait=keep, on_update=list(si.on_update or [])
                    )
                new_insts.append(inst)
            bb.instructions[:] = new_insts


def build_nc(n_rep: int = 1, x_bufs: int = 4, out_split: bool = True,
             dummies: bool = True, last_kspl: int = 2, dual: bool = False,
             dummy_n: int = 10, pre_dummy_n: int = 30, out_bf16: bool = False,
             bank_alt: bool = False):
    """Build the per-core Bass program. n_rep > 1 wraps the computation in a
    hardware For_i loop (same I/O, output overwritten) so harnesses can
    measure steady-state HW time by wall-clock slope; grading uses n_rep=1.

    dummies: fill PE idle gaps between s-slices with throwaway matmuls into a
      scratch PSUM bank so the Tensor engine never idles and stays at its max
      pstate (it runs at ~2x cycle time until ~3 us of CONTINUOUS execution;
      DMA-paced bursts of ~3.4 us never ramp up otherwise).
    last_kspl: split the last s-slice's DMA into this many K-range pieces so
      its matmuls overlap the stream and only the last piece is a PE tail.
    dual: alternate s-slice DMAs between the SP and Act HWDGE queues.
    """
    SS = S // J
    ODT = BF16 if out_bf16 else F32
    nc = bass.Bass("TRN2", target_bir_lowering=False, debug=False)
    x_ap = nc.dram_tensor("xh", [J, P, KO * SS], BF16, kind="ExternalInput").ap()
    w_ap = nc.dram_tensor("wt", [P, KO, OUT], BF16, kind="ExternalInput").ap()
    o_ap = nc.dram_tensor("out", [OUT, S], ODT, kind="ExternalOutput").ap()

    with tile.TileContext(nc) as tc:
        with (
            tc.tile_pool(name="wpool", bufs=1) as wpool,
            tc.tile_pool(name="xpool", bufs=x_bufs) as xpool,
            tc.tile_pool(name="opool", bufs=1) as opool,
            tc.tile_pool(name="pspool", bufs=2, space="PSUM") as pspool,
            tc.tile_pool(name="psbpool", bufs=2, space="PSUM") as psbpool,
            tc.tile_pool(name="jkpool", bufs=1, space="PSUM") as jkpool,
        ):
            w_sb = wpool.tile([P, KO, OUT], BF16)
            # SWDGE ring for the 0.5 MB weight preload so the x stream starts
            # immediately on the qSP HWDGE ring.
            nc.gpsimd.dma_start(w_sb[:], w_ap[:])
            ps_junk = jkpool.tile([OUT, 512], F32, tag="junk")
            # dedicated 2D moving operand for dummy matmuls (N=512 so each
            # dummy is a ~213 ns solid burst on the PE, not issue-gap noise)
            dmv = wpool.tile([P, 512], BF16)
            nc.gpsimd.dma_start(dmv[:], w_ap[:, 0:8, :])

            def emit_dummies(n):
                # reads of resident tiles; write-only scratch PSUM; no
                # cross-engine deps, so these run back-to-back on the PE
                # filling what would otherwise be an idle (pstate-dropping)
                # gap, and the next real matmul preempts them at the next
                # instruction boundary.
                for d in range(n):
                    nc.tensor.matmul(
                        ps_junk[:, :512], w_sb[:, 0, :], dmv[:, :],
                        start=True, stop=True, skip_group_check=True,
                    )

            def body():
                osb = opool.tile([OUT, S], ODT, tag="osb")
                if dummies:
                    emit_dummies(pre_dummy_n)
                for j in range(J):
                    xt = xpool.tile([P, KO * SS], BF16, tag="xh")
                    nspl = last_kspl if j == J - 1 else 1
                    kh = KO // nspl
                    eng = nc.scalar if (dual and j % 2) else nc.sync
                    for h in range(nspl):
                        eng.dma_start(
                            xt[:, h * kh * SS:(h + 1) * kh * SS],
                            x_ap[j][:, h * kh * SS:(h + 1) * kh * SS],
                        )
                    ps = pspool.tile([OUT, 512], F32, tag="ps")
                    if bank_alt:
                        # ping-pong even/odd K-chunks between two PSUM banks
                        # so no two consecutive PE instructions accumulate
                        # into the same bank (same-bank back-to-back matmuls
                        # stall on accumulator turnaround); fold A+B below.
                        psb = psbpool.tile([OUT, 512], F32, tag="psb")
                        for ko in range(KO):
                            tgt = ps if ko % 2 == 0 else psb
                            nc.tensor.matmul(
                                tgt[:, :SS], w_sb[:, ko, :],
                                xt[:, ko * SS:(ko + 1) * SS],
                                start=(ko < 2), stop=(ko >= KO - 2),
                                skip_group_check=True,
                            )
                        nc.scalar.copy(osb[:, j * SS:(j + 1) * SS], ps[:, :SS])
                        nc.vector.tensor_add(
                            osb[:, j * SS:(j + 1) * SS],
                            osb[:, j * SS:(j + 1) * SS], psb[:, :SS],
                        )
                    else:
                        for ko in range(KO):
                            nc.tensor.matmul(
                                ps[:, :SS], w_sb[:, ko, :],
                                xt[:, ko * SS:(ko + 1) * SS],
                                start=(ko == 0), stop=(ko == KO - 1),
                                skip_group_check=True,
                            )
                        nc.scalar.copy(osb[:, j * SS:(j + 1) * SS], ps[:, :SS])
                    if dummies and j < J - 1:
                        emit_dummies(dummy_n)
                    # overlap most of the output store with the last slice's
                    # matmuls; only the final SS columns are a tail. (SWDGE
                    # can't encode a strided DRAM dst - "ISA wrong length" -
                    # so these ride the Act HWDGE queue.)
                    if out_split and j == J - 2:
                        nc.scalar.dma_start(
                            o_ap[:, :(J - 1) * SS], osb[:, :(J - 1) * SS]
                        )
                if out_split:
                    nc.scalar.dma_start(
                        o_ap[:, (J - 1) * SS:], osb[:, (J - 1) * SS:]
                    )
                else:
                    nc.scalar.dma_start(o_ap[:], osb[:])

            if n_rep == 1:
                body()
            elif n_rep <= 4:
                # unrolled (TimelineSim can't follow For_i register branches)
                for _ in range(n_rep):
                    body()
            else:
                with tc.For_i(0, n_rep, 1):
                    body()
    _split_sync_waits(nc)
    return nc


def make_in_maps(x: np.ndarray, adapter_ids: np.ndarray, weight: np.ndarray):
    """Host-side sharding: per-core adapter gather + bf16 round + s-major tiling.

    xh[j, p, ko*SS + s] = bf16(x[b, j*SS + s, ko*128 + p])
    wt[p, ko, o]        = bf16(weight[id_b, o, ko*128 + p])
    """
    SS = S // J
    x = np.asarray(x, dtype=np.float32)
    ids = np.asarray(adapter_ids).astype(np.int64)
    w = np.asarray(weight, dtype=np.float32)

    xa = np.ascontiguousarray(x.transpose(0, 2, 1))        # [B, IN, S]
    xh = xa.astype(ml_dtypes.bfloat16).reshape(B, KO, P, J, SS)
    xh = np.ascontiguousarray(xh.transpose(0, 3, 2, 1, 4)) # [B, J, P, KO, SS]
    xh = xh.reshape(B, J, P, KO * SS)

    wsel = w[ids]                                          # [B, OUT, IN]
    wt = np.ascontiguousarray(wsel.transpose(0, 2, 1)).reshape(B, KO, P, OUT)
    wt = np.ascontiguousarray(wt.transpose(0, 2, 1, 3))    # [B, P, KO, OUT]
    wh = wt.astype(ml_dtypes.bfloat16)

    return [{"xh": xh[b], "wt": wh[b]} for b in range(B)]


_NC_CACHE = {}


def kernel(x, adapter_ids, weight):
    x = np.asarray(x)
    assert x.shape == (B, S, IN), x.shape
    if "nc" not in _NC_CACHE:
        _NC_CACHE["nc"] = build_nc()
    nc = _NC_CACHE["nc"]
    in_maps = make_in_maps(x, adapter_ids, weight)
    res = bass_utils.run_bass_kernel_spmd(
        nc, in_maps, core_ids=list(range(N_CORES)), trace=False
    )
    out = np.stack(
        [res.results[b]["out"].T for b in range(B)], axis=0
    )
    return np.ascontiguousarray(out, dtype=np.float32)
